# revision 1
# baseline (speedup 1.0000x reference)
"""Trainium2 Bass kernel for nn_MultiHeadDotProductAttention_24756191494231.

Masked (toeplitz-structured) linear attention:
    q = relu(query/8); k = relu(key)
    attn = (q @ k^T) * |toeplitz_mask| ; attn /= attn.sum(-1) ; out = attn @ v

Sharding: 8 cores = 2 batch-groups (4 batches) x 4 head-groups (3 heads).
Each core computes 12 (batch, head) pairs.

Device pipeline per (head, batch):
  S^T[k,l] = K'^T.T @ Q'^T      (bf16 matmuls, PSUM f32, k-chunks of <=121)
  A[k,l]   = S^T * |mask^T|     (tensor_tensor; mask read from a
                                 shift-replicated params tile via a strided AP
                                 -- the toeplitz gather becomes pure layout)
  O[l,:]   = A.T @ [V|1]        (bf16; ones column gives Z = row-sum)
  out      = O[:, :64] / Z      (reciprocal + broadcast multiply)

The mask operand tile ("mop") holds |params| shifted by s(r) = r%24 + 48*(r//24)
per partition; one AP with dims [[48,24],[1,24]] then reads mask^T rows for a
whole 120-row chunk. The shifts are materialized by a single DRAM->DRAM DMA
whose output access pattern is a parallelogram (affine in flat DRAM space).

Mask application is split across engines per k-chunk: chunk 0 goes straight
through DVE from PSUM; chunks 1-4 are copied PSUM->SBUF(bf16) on ScalarE, then
multiplied in-place in bf16 2x mode on DVE (chunks 1,3,4) or GpSimd (chunk 2).
"""
import sys

for _p in ("/opt/trn_rl_repo", "/root/.axon_site/_ro/trn_rl_repo"):
    if _p not in sys.path:
        sys.path.insert(0, _p)

import numpy as np
import ml_dtypes

NBX = NBY = 24
B, H, D = 8, 12, 64
L = NBX * NBY + 1          # 577
LP = 578                   # A-tile chunk stride (even => bf16 runs 4B-aligned)
NB = 4                     # batches per core
NH = 3                     # heads per core
CNT = [121, 120, 120, 120, 96]       # k-chunk sizes (CLS + 24-aligned grid)
KS = [0, 121, 241, 361, 481]         # k-chunk starts (in l index)
LW = [128, 128, 128, 128, 65]        # l-chunk sizes
MOPW = 2520                          # stage row width (2304 + max shift 215 + 1)
CLSW = 1128                          # mop_cls width (f in [1176, 2304))
MAINW = 1848                         # mop_main width (f in [216, 2064))

_CACHE = {}


def _split_excess_waits(nc):
    """Walrus in this toolchain accepts at most ONE sync-wait per instruction
    (zero on Pool-engine ops). Move excess waits onto same-engine
    InstEventSemaphore instructions inserted immediately before the offending
    instruction; engines execute in order, so semantics are unchanged."""
    import concourse.mybir as mb
    ctr = 0
    f = nc.m.functions[0]
    for bb in f.blocks:
        insts = list(bb.instructions)
        out = []
        changed = False
        for inst in insts:
            si = inst.sync_info
            keep = 0 if inst.engine == mb.EngineType.Pool else 1
            if si is not None and len(si.on_wait) > keep:
                waits = list(si.on_wait)
                moved = waits[:-keep] if keep else waits
                kept = waits[-keep:] if keep else []
                for w in moved:
                    ctr += 1
                    ev = mb.InstEventSemaphore(
                        name=f"zz_waitsplit_{ctr}", ins=[], outs=[])
                    ev.engine = inst.engine
                    ev.sync_info = mb.SyncInfo(on_wait=[w], on_update=[])
                    out.append(ev)
                inst.sync_info = mb.SyncInfo(
                    on_wait=kept, on_update=list(si.on_update))
                changed = True
            out.append(inst)
        if changed:
            bb.instructions = out


def _build_bass():
    import concourse.bass as bass
    import concourse.mybir as mybir
    from concourse.bass_types import AP
    from concourse.tile import TileContext

    F32 = mybir.dt.float32
    BF16 = mybir.dt.bfloat16
    Alu = mybir.AluOpType
    Act = mybir.ActivationFunctionType

    nc = bass.Bass("TRN2")
    qkv_d = nc.dram_tensor("qkv", (NH, 128, 4 * L + NB * 325), F32,
                           kind="ExternalInput")
    prm_d = nc.dram_tensor("prm", (128, 54), F32, kind="ExternalInput")
    ones_d = nc.dram_tensor("ones1", (1, CLSW), BF16, kind="ExternalInput")
    o_d = nc.dram_tensor("o", (NH, NB, 128, 320), F32, kind="ExternalOutput")

    with TileContext(nc) as tc:
        with (
            tc.tile_pool(name="sb", bufs=2) as sb,
            tc.tile_pool(name="sb3", bufs=3) as sb3,
            tc.tile_pool(name="sb1", bufs=1) as sb1,
            tc.tile_pool(name="ps", bufs=3, space="PSUM") as ps,
            tc.tile_pool(name="ps_o", bufs=2, space="PSUM") as ps_o,
            tc.tile_pool(name="dram", bufs=1, space="DRAM") as dr,
        ):
            # ---- |params| -> DRAM scratch (flat, per-head contiguous) ----
            prm_sb = sb1.tile([128, 54], F32)
            nc.sync.dma_start(prm_sb, prm_d[:, :])
            prm_abs = sb1.tile([128, 54], F32)
            nc.scalar.activation(prm_abs, prm_sb, Act.Abs)
            scratch = dr.tile([NH * 2304], F32, tag="scratch")
            nc.sync.dma_start(AP(scratch.tensor, 0, [[54, 128], [1, 54]]), prm_abs)

            def load_head(h):
                # ---- mask operand tiles ----
                stage = dr.tile([121 * MOPW + 64], BF16, tag=f"stage{h}")
                out_ap = AP(stage.tensor, MOPW,
                            [[24 * MOPW + 48, 5], [MOPW + 1, 24], [1, 2304]])
                in_ap = AP(scratch.tensor, h * 2304, [[0, 5], [0, 24], [1, 2304]])
                nc.gpsimd.dma_start(out_ap, in_ap)   # SWDGE: cast + step-0 src

                # mop_cls[p, y] = stage[p, 1176 + y]; row 0 then overwritten
                # with ones (CLS mask row) via a direct HWDGE load
                mop_cls = sb.tile([121, CLSW], BF16, tag="mop_cls")
                nc.sync.dma_start(mop_cls,
                                  AP(stage.tensor, 1176, [[MOPW, 121], [1, CLSW]]))
                nc.sync.dma_start(mop_cls[0:1, :], ones_d[:, :])
                # mop_main[p, y] = stage[p + 1, 216 + y]
                mop_main = sb.tile([120, MAINW], BF16, tag="mop_main")
                nc.sync.dma_start(mop_main,
                                  AP(stage.tensor, MOPW + 216, [[MOPW, 120], [1, MAINW]]))

                # ---- Q|K|V in one SWDGE cast-load, then relu (4x mode) ----
                qkv_r = sb.tile([128, 4 * L + NB * 325], BF16, tag="qkv_r")
                nc.gpsimd.dma_start(qkv_r, qkv_d[h])
                qT_b = sb.tile([128, 2 * L], BF16, tag="qT_b")
                nc.vector.tensor_scalar(out=qT_b, in0=qkv_r[:, 0:2 * L],
                                        scalar1=0.125, scalar2=0.0,
                                        op0=Alu.mult, op1=Alu.max)
                kT_b = sb.tile([128, 2 * L], BF16, tag="kT_b")
                nc.vector.tensor_scalar(out=kT_b, in0=qkv_r[:, 2 * L:4 * L],
                                        scalar1=0.0, scalar2=None, op0=Alu.max)
                o_sb = sb.tile([128, NB * 320], F32, tag="o_sb")
                return dict(mop_cls=mop_cls, mop_main=mop_main, qkv=qkv_r,
                            qT=qT_b, kT=kT_b, o_sb=o_sb, h=h)

            def mask_chunk(R, c, s_ps, a_t):
                cnt = CNT[c]
                co = LP * c + 1               # a_t column of l=0 for chunk c
                if c == 0:
                    # direct: TT from PSUM + separate CLS-query col
                    nc.vector.tensor_copy(a_t[0:cnt, co:co + 1],
                                          s_ps[0:cnt, 0:1])
                    in1 = AP(R["mop_cls"].tensor, 0,
                             [[CLSW, 121], [48, 24], [1, 24]])
                    in0 = s_ps[0:cnt, 1:L].rearrange("p (i j) -> p i j", j=24)
                    outap = a_t[0:cnt, co + 1:co + L].rearrange(
                        "p (i j) -> p i j", j=24)
                    nc.vector.tensor_tensor(out=outap, in0=in0,
                                            in1=in1, op=Alu.mult)
                else:
                    # copy all 577 cols to bf16 on ScalarE, then
                    # multiply grid cols in place (2x bf16)
                    nc.scalar.activation(a_t[0:cnt, co:co + L],
                                         s_ps[0:cnt, 0:L], Act.Copy)
                    off = 48 * (20 - 5 * c)
                    in1 = AP(R["mop_main"].tensor, off,
                             [[MAINW, cnt], [48, 24], [1, 24]])
                    io = a_t[0:cnt, co + 1:co + L].rearrange(
                        "p (i j) -> p i j", j=24)
                    eng = nc.gpsimd if c == 2 else nc.vector
                    eng.tensor_tensor(out=io, in0=io, in1=in1, op=Alu.mult)

            def build_pair(R, b):
                # S^T matmuls + mask application for one (head, batch) pair
                pr = 64 * (b // 2)            # partition row of this batch pair
                xo = L * (b % 2)              # column offset within the pair
                a_t = sb3.tile([128, 5 * LP], BF16, tag="a_t")
                for c in range(5):
                    cnt = CNT[c]
                    s_ps = ps.tile([128, L], F32, tag="s_ps")
                    lhs = R["kT"][pr:pr + 64, xo + KS[c]:xo + KS[c] + cnt]
                    nc.tensor.matmul(s_ps[0:cnt, 0:512], lhs,
                                     R["qT"][pr:pr + 64, xo:xo + 512],
                                     start=True, stop=True)
                    nc.tensor.matmul(s_ps[0:cnt, 512:577], lhs,
                                     R["qT"][pr:pr + 64, xo + 512:xo + 577],
                                     start=True, stop=True)
                    mask_chunk(R, c, s_ps, a_t)
                return a_t

            def finish_pair(R, b, a_t):
                # A.T @ [V|1], normalize, and store when the head completes
                o_ps = ps_o.tile([128, 325], F32, tag="o_ps")
                for lc in range(5):
                    lw = LW[lc]
                    for c in range(5):
                        nc.tensor.matmul(
                            o_ps[0:lw, 65 * lc:65 * lc + 65],
                            a_t[0:CNT[c], LP * c + 1 + 128 * lc:LP * c + 1 + 128 * lc + lw],
                            R["qkv"][0:CNT[c], 4 * L + 325 * b + 65 * c:4 * L + 325 * b + 65 * c + 65],
                            start=(c == 0), stop=(c == 4))

                rz = sb.tile([128, 5], F32, tag="rz")
                zin = o_ps[:, :].rearrange("p (c d) -> p c d", d=65)[:, :, 64:65]
                nc.vector.reciprocal(rz[:, :].rearrange("p (c d) -> p c d", d=1), zin)
                in0 = o_ps[:, :].rearrange("p (c d) -> p c d", d=65)[:, :, 0:64]
                in1 = AP(rz.tensor, 0, [[5, 128], [1, 5], [0, 64]])
                nc.vector.tensor_tensor(
                    out=R["o_sb"][:, 320 * b:320 * b + 320].rearrange(
                        "p (c d) -> p c d", d=64),
                    in0=in0, in1=in1, op=Alu.mult)
                if b == NB - 1:
                    nc.sync.dma_start(
                        AP(o_d, R["h"] * NB * 128 * 320,
                           [[320, 128], [128 * 320, NB], [1, 320]]),
                        R["o_sb"])

            # software pipeline: masks for pair i overlap AV of pair i-1
            pending = None
            for h in range(NH):
                R = load_head(h)
                for b in range(NB):
                    a_t = build_pair(R, b)
                    if pending is not None:
                        finish_pair(*pending)
                    pending = (R, b, a_t)
            finish_pair(*pending)

    _split_excess_waits(nc)
    return nc


def _get_nc():
    if "nc" not in _CACHE:
        _CACHE["nc"] = _build_bass()
    return _CACHE["nc"]


def _host_shard(query, key, value, topological_params):
    """Build the 8 per-core input dicts (pure slicing / layout transforms)."""
    in_maps = []
    q = np.asarray(query, dtype=np.float32)
    k = np.asarray(key, dtype=np.float32)
    v = np.asarray(value, dtype=np.float32)
    p = np.asarray(topological_params, dtype=np.float32)
    ones1 = np.ones((1, CLSW), dtype=ml_dtypes.bfloat16)
    for u in range(2):            # batch group
        for g in range(4):        # head group
            bs = slice(4 * u, 4 * u + 4)
            hs = slice(3 * g, 3 * g + 3)

            def pack_T(x):
                # [4b, L, 3h, 64] -> [3h, 128p, 2*L]; p = d + 64*(b//2),
                # col = (b%2)*L + l
                t = x[bs, :, hs, :]                       # [4, L, 3, 64]
                t = t.transpose(2, 0, 3, 1)               # [3, 4, 64, L]
                t = t.reshape(3, 2, 2, 64, L)             # [3, bhi, blo, d, L]
                t = t.transpose(0, 1, 3, 2, 4)            # [3, bhi, d, blo, L]
                return np.ascontiguousarray(t.reshape(3, 128, 2 * L))

            vs = v[bs, :, hs, :]                          # [4, L, 3, 64]
            v_r = np.zeros((3, 128, NB, 5, 65), np.float32)
            for c in range(5):
                n = CNT[c]
                blk = vs[:, KS[c]:KS[c] + n].transpose(2, 1, 0, 3)
                v_r[:, :n, :, c, 0:64] = blk
                v_r[:, :n, :, c, 64] = 1.0
            qkv = np.concatenate(
                [pack_T(q), pack_T(k), v_r.reshape(3, 128, NB * 325)], axis=2)
            prm = np.ascontiguousarray(p[hs]).reshape(128, 54)
            in_maps.append({
                "qkv": np.ascontiguousarray(qkv),
                "prm": prm,
                "ones1": ones1,
            })
    return in_maps


def kernel(query, key, value, topological_params):
    from concourse import bass_utils
    nc = _get_nc()
    in_maps = _host_shard(query, key, value, topological_params)
    res = bass_utils.run_bass_kernel_spmd(nc, in_maps, core_ids=list(range(8)))
    out = np.empty((B, L, H, D), dtype=np.float32)
    for u in range(2):
        for g in range(4):
            o = res.results[4 * u + g]["o"]          # [3, 4, 128, 320]
            o = o.reshape(3, 4, 128, 5, 64)
            for lc in range(5):
                lw = LW[lc]
                blk = o[:, :, 0:lw, lc, :]           # [3, 4, lw, 64]
                out[4 * u:4 * u + 4, 128 * lc:128 * lc + lw, 3 * g:3 * g + 3, :] = \
                    blk.transpose(1, 2, 0, 3)
    return out



# revision 2
# speedup vs baseline: 1.0330x; 1.0330x over previous
"""Trainium2 Bass kernel v2 for nn_MultiHeadDotProductAttention_24756191494231.

Masked (toeplitz-structured) linear attention:
    q = relu(query/8); k = relu(key)
    attn = (q @ k^T) * |toeplitz_mask| ; attn /= attn.sum(-1) ; out = attn @ v

Sharding: 8 cores = 2 batch-groups (4 batches) x 4 head-groups (3 heads).
Each core computes 12 (batch, head) pairs.

v2 restructure (vs v1 baseline):
  - relu/scale/cast and the |mask| gather are done on the HOST; the device
    receives bf16 q^T/k^T/v and fully materialized per-chunk mask tiles.
  - S^T[k,q] bf16 matmuls in 5 key-chunks of [128,128,128,128,65]; the
    512-wide part goes to a single-bank PSUM tile (deep rotation), the
    65-wide query tail of all 5 chunks accumulates in one shared bank and
    is masked by ONE merged TT per pair.
  - mask apply per (pair, chunk) via one of three engine paths (tunable
    METH table): Act copy->DVE 2x TT, Act copy->Pool TT, or direct DVE 1x
    TT from PSUM.
  - AV in A-orientation: out[l-chunk, 65] (64 v cols + ones col = Z).
  - One Act evacuation (PSUM->SBUF) per pair; normalize (O/Z) on the host.
"""
import sys

for _p in ("/opt/trn_rl_repo", "/root/.axon_site/_ro/trn_rl_repo"):
    if _p not in sys.path:
        sys.path.insert(0, _p)

import numpy as np
import ml_dtypes

NBX = NBY = 24
B, H, D = 8, 12, 64
L = NBX * NBY + 1          # 577
NB = 4                     # batches per core
NH = 3                     # heads per core
CNT = [128, 128, 128, 128, 65]       # key-chunk sizes
KS = [0, 128, 256, 384, 512]         # key-chunk starts
LW = [128, 128, 128, 128, 65]        # l(query)-chunk sizes
CW = 580                             # padded column stride (4B aligned bf16)
QW = 640                             # q/k block stride (chunk-4 zero padding)
MTW = 5 * 65                         # tail-mask columns

# mask-apply method per (head, batch, chunk) for the 512-wide part:
#   0 = Act copy -> DVE 2x in-place TT   (Act ~612ns, DVE ~326ns)
#   1 = Act copy -> Pool in-place TT     (Act ~612ns, Pool ~1206ns)
#   2 = direct DVE 1x TT from PSUM       (DVE ~658ns)
_PAT_A = (2, 1, 0, 1, 2)
_PAT_B = (2, 1, 0, 1, 2)
_PAT_L = (1, 1, 0, 2, 2)   # last pair: spread for fast drain
METH = [[_PAT_A, _PAT_A, _PAT_A, _PAT_B] for _ in range(NH)]
METH[NH - 1][NB - 1] = _PAT_L
BUFS = dict(sb=3, sba=5, sbo=6, sa=5, st=1, po=2)
AVDEPTH = 3
EVAC_DVE = {3, 6, 9}   # pair indices whose evac runs on DVE
MODE = 'full'   # 'full' | 'skeleton' (no mask stage, AV reads m_t)
FP8 = True     # fp8e4 DoubleRow S^T matmuls (q/k in fp8, halves PE cost)
EVAC2 = False  # interp-safe evacuation (reads only written PSUM bytes)


def _av_order(pat):
    """AV accumulation order: Pool-masked (meth==1) chunks last."""
    return [c for c in range(5) if pat[c] != 1] +            [c for c in range(5) if pat[c] == 1]

_CACHE = {}


def _split_excess_waits(nc):
    """Walrus accepts at most ONE sync-wait per instruction (zero on
    Pool-engine ops). Move excess waits onto same-engine InstEventSemaphore
    instructions inserted immediately before the offending instruction."""
    import concourse.mybir as mb
    ctr = 0
    f = nc.m.functions[0]
    for bb in f.blocks:
        insts = list(bb.instructions)
        out = []
        changed = False
        for inst in insts:
            si = inst.sync_info
            keep = 0 if inst.engine == mb.EngineType.Pool else 1
            if si is not None and len(si.on_wait) > keep:
                waits = list(si.on_wait)
                moved = waits[:-keep] if keep else waits
                kept = waits[-keep:] if keep else []
                for w in moved:
                    ctr += 1
                    ev = mb.InstEventSemaphore(
                        name=f"zz_waitsplit_{ctr}", ins=[], outs=[])
                    ev.engine = inst.engine
                    ev.sync_info = mb.SyncInfo(on_wait=[w], on_update=[])
                    out.append(ev)
                inst.sync_info = mb.SyncInfo(
                    on_wait=kept, on_update=list(si.on_update))
                changed = True
            out.append(inst)
        if changed:
            bb.instructions = out


def _build_bass(split_waits=True):
    import concourse.bass as bass
    import concourse.mybir as mybir
    from concourse.bass_types import AP
    from concourse.tile import TileContext

    F32 = mybir.dt.float32
    BF16 = mybir.dt.bfloat16
    F16 = mybir.dt.float16
    Alu = mybir.AluOpType
    Act = mybir.ActivationFunctionType

    F8 = mybir.dt.float8e4
    nc = bass.Bass("TRN2")
    if FP8:
        qk_d = nc.dram_tensor("qk", (NH, 32, 4 * NB * QW), F8,
                              kind="ExternalInput")
    else:
        qk_d = nc.dram_tensor("qk", (NH, 64, 2 * NB * CW), BF16,
                              kind="ExternalInput")
    v_d = nc.dram_tensor("v", (NH, 128, NB * 330), F16, kind="ExternalInput")
    m_d = nc.dram_tensor("m", (NH, 128, 5 * CW + MTW), F16,
                         kind="ExternalInput")
    o_d = nc.dram_tensor("o", (NH, NB, 128, 330), F16,
                         kind="ExternalOutput")

    with TileContext(nc) as tc:
        with (
            tc.tile_pool(name="sb", bufs=BUFS["sb"]) as sb,
            tc.tile_pool(name="sba", bufs=BUFS["sba"]) as sba,
            tc.tile_pool(name="sbo", bufs=BUFS["sbo"]) as sbo,
            tc.tile_pool(name="ps_a", bufs=BUFS["sa"], space="PSUM") as ps_a,
            tc.tile_pool(name="ps_t", bufs=BUFS["st"], space="PSUM") as ps_t,
            tc.tile_pool(name="ps_o", bufs=BUFS["po"], space="PSUM") as ps_o,
        ):
            def load_head(h):
                if FP8:
                    qk = sb.tile([32, 4 * NB * QW], F8, tag="qk")
                    half = 2 * NB * QW
                    nc.sync.dma_start(qk[:, 0:half], qk_d[h][:, 0:half])
                    m_t = sb.tile([128, 5 * CW + MTW], F16, tag="m_t")
                    nc.sync.dma_start(m_t[:, 0:2 * CW], m_d[h][:, 0:2 * CW])
                    nc.sync.dma_start(qk[:, half:2 * half],
                                      qk_d[h][:, half:2 * half])
                    nc.sync.dma_start(m_t[:, 2 * CW:5 * CW + MTW],
                                      m_d[h][:, 2 * CW:5 * CW + MTW])
                    v_t = sb.tile([128, NB * 330], F16, tag="v_t")
                    nc.sync.dma_start(v_t, v_d[h])
                    return dict(qk=qk, v=v_t, m=m_t, h=h)
                qk = sb.tile([64, 2 * NB * CW], BF16, tag="qk")
                nc.sync.dma_start(qk[:, 0:4 * CW], qk_d[h][:, 0:4 * CW])
                m_t = sb.tile([128, 5 * CW + MTW], F16, tag="m_t")
                nc.sync.dma_start(m_t[:, 0:2 * CW], m_d[h][:, 0:2 * CW])
                nc.sync.dma_start(qk[:, 4 * CW:8 * CW],
                                  qk_d[h][:, 4 * CW:8 * CW])
                nc.sync.dma_start(m_t[:, 2 * CW:5 * CW + MTW],
                                  m_d[h][:, 2 * CW:5 * CW + MTW])
                v_t = sb.tile([128, NB * 330], F16, tag="v_t")
                nc.sync.dma_start(v_t, v_d[h])
                return dict(qk=qk, v=v_t, m=m_t, h=h)

            def av_lcgroup(R, b, a_t, o_ps, lc):
                # one sequential PSUM accumulation group (all 5 key chunks)
                lw = LW[lc]
                src_t = R["m"] if MODE == 'skeleton' else a_t
                for c in range(5):
                    cnt = CNT[c]
                    nc.tensor.matmul(
                        o_ps[0:lw, 66 * lc:66 * lc + 65],
                        src_t[0:cnt, CW * c + 128 * lc:CW * c + 128 * lc + lw],
                        R["v"][0:cnt, 330 * b + 66 * c:330 * b + 66 * c + 65],
                        start=(c == 0), stop=(c == 4))

            def evac_pair(R, b, o_ps, eng="act"):
                # fp16 output with 1/16 scale; the scale cancels in the
                # host-side O/Z divide
                o_sb = sbo.tile([128, 330], F16, tag="o_sb")
                if eng == "dve" and not EVAC2:
                    nc.vector.tensor_scalar(out=o_sb, in0=o_ps[:, 0:330],
                                            scalar1=0.0625, scalar2=None,
                                            op0=Alu.mult)
                    nc.sync.dma_start(o_d[R["h"], b], o_sb)
                    return
                if EVAC2:
                    nc.scalar.activation(
                        o_sb[:, 0:264].rearrange(
                            "p (l j) -> p l j", j=66)[:, :, 0:65],
                        o_ps[:, 0:264].rearrange(
                            "p (l j) -> p l j", j=66)[:, :, 0:65],
                        Act.Copy, scale=0.0625)
                    nc.scalar.activation(o_sb[0:65, 264:329],
                                         o_ps[0:65, 264:329], Act.Copy,
                                         scale=0.0625)
                    dst = o_d[R["h"], b]
                    nc.sync.dma_start(
                        dst[:, 0:264].rearrange(
                            "p (l j) -> p l j", j=66)[:, :, 0:65],
                        o_sb[:, 0:264].rearrange(
                            "p (l j) -> p l j", j=66)[:, :, 0:65])
                    nc.sync.dma_start(dst[0:65, 264:329],
                                      o_sb[0:65, 264:329])
                else:
                    nc.scalar.activation(o_sb, o_ps[:, 0:330], Act.Copy,
                                         scale=0.0625)
                    nc.sync.dma_start(o_d[R["h"], b], o_sb)

            # software pipeline: S^T+mask of pair i overlaps AV of pair
            # i-AVDEPTH (deeper pipelining decouples mask latency from PE)
            heads = [None, None, None]
            heads[0] = load_head(0)
            heads[1] = load_head(1)
            pending = []
            for h in range(NH):
                R = heads[h]
                for b in range(NB):
                    hh = R["h"]
                    if FP8:
                        qo = 4 * QW * b
                        ko = qo + 2 * QW
                        pitch = 4 * NB * QW
                    else:
                        qo = 2 * CW * b
                        ko = 2 * CW * b + CW
                    a_t = sba.tile([128, 5 * CW], F16, tag="a_t")
                    s_t = ps_t.tile([128, MTW], F32, tag="s_t")
                    idx = h * NB + b
                    npop = 0
                    if len(pending) >= AVDEPTH:
                        npop = 1
                    if idx >= NH * NB - (AVDEPTH - 1) and pending:
                        npop = min(2, len(pending))
                    readies = [pending.pop(0) for _ in range(npop)]
                    ready = readies[0] if readies else None
                    if ready is not None:
                        o_ps = ps_o.tile([128, 330], F32, tag="o_ps")
                    for c in range(5):
                        cnt = CNT[c]
                        s_a = ps_a.tile([128, 512], F32, tag="s_a")
                        if FP8:
                            cmm = 128 if c == 4 else cnt  # pad-keys: full M
                            qkt = R["qk"].tensor
                            lhs = AP(qkt, ko + KS[c],
                                     [[pitch, 32], [QW, 2], [1, cmm]])
                            rhs_a = AP(qkt, qo,
                                       [[pitch, 32], [QW, 2], [1, 512]])
                            rhs_t = AP(qkt, qo + 512,
                                       [[pitch, 32], [QW, 2], [1, 65]])
                            dr = mybir.MatmulPerfMode.DoubleRow
                            nc.tensor.matmul(s_a[0:cmm, 0:512], lhs, rhs_a,
                                             start=True, stop=True,
                                             perf_mode=dr)
                            nc.tensor.matmul(s_t[0:cmm, 65 * c:65 * c + 65],
                                             lhs, rhs_t,
                                             start=True, stop=True,
                                             perf_mode=dr,
                                             skip_group_check=True)
                        else:
                            lhs = R["qk"][:, ko + KS[c]:ko + KS[c] + cnt]
                            nc.tensor.matmul(s_a[0:cnt, 0:512],
                                             lhs, R["qk"][:, qo:qo + 512],
                                             start=True, stop=True)
                            nc.tensor.matmul(s_t[0:cnt, 65 * c:65 * c + 65],
                                             lhs,
                                             R["qk"][:, qo + 512:qo + 577],
                                             start=True, stop=True,
                                             skip_group_check=True)

                        if MODE == 'skeleton':
                            continue
                        meth = METH[hh][b][c]
                        ao = a_t[0:cnt, CW * c:CW * c + 512]
                        mo = R["m"][0:cnt, CW * c:CW * c + 512]
                        if meth == 2:
                            nc.vector.tensor_tensor(
                                out=ao, in0=s_a[0:cnt, 0:512], in1=mo,
                                op=Alu.mult)
                        else:
                            nc.scalar.activation(ao, s_a[0:cnt, 0:512],
                                                 Act.Copy)
                            eng = nc.vector if meth == 0 else nc.gpsimd
                            eng.tensor_tensor(out=ao, in0=ao, in1=mo,
                                              op=Alu.mult)
                    if ready is not None:
                        pR, pb, pa = ready
                        for lc in range(5):
                            av_lcgroup(pR, pb, pa, o_ps, lc)
                    if MODE != 'skeleton':
                        # merged query-tail mask TT for all 5 chunks
                        ta = AP(a_t.tensor, 512,
                                [[5 * CW, 128], [CW, 5], [1, 65]])
                        nc.vector.tensor_tensor(
                            out=ta,
                            in0=s_t[:, :].rearrange("p (c j) -> p c j", j=65),
                            in1=R["m"][:, 5 * CW:5 * CW + MTW].rearrange(
                                "p (c j) -> p c j", j=65),
                            op=Alu.mult)
                    if ready is not None:
                        evac_pair(ready[0], ready[1], o_ps,
                                  eng="dve" if idx in EVAC_DVE else "act")
                    for extra in readies[1:]:
                        o_ps2 = ps_o.tile([128, 330], F32, tag="o_ps")
                        for lc in range(5):
                            av_lcgroup(extra[0], extra[1], extra[2], o_ps2, lc)
                        evac_pair(extra[0], extra[1], o_ps2)
                    pending.append((R, b, a_t))
                if h + 2 < NH:
                    heads[h + 2] = load_head(h + 2)
            # drain remaining pairs
            for ready in pending:
                pR, pb, pa = ready
                o_ps = ps_o.tile([128, 330], F32, tag="o_ps")
                for lc in range(5):
                    av_lcgroup(pR, pb, pa, o_ps, lc)
                evac_pair(pR, pb, o_ps)

    if split_waits:
        _split_excess_waits(nc)
    return nc


def _get_nc():
    if "nc" not in _CACHE:
        _CACHE["nc"] = _build_bass()
    return _CACHE["nc"]


def _dist_index():
    """Flattened toeplitz displacement index [L-1, L-1] into params[:, 4*NBX*NBY]."""
    gi = np.arange(NBX)
    dist = ((gi[:, None, None, None] - gi[None, None, :, None] + NBX) * 2 * NBY
            + gi[None, :, None, None] - gi[None, None, None, :] + NBY)
    return dist.reshape(NBX * NBY, NBX * NBY)


_DIST = _dist_index()


def _host_shard(query, key, value, topological_params):
    """Build the 8 per-core input dicts (slicing / relu / cast / mask)."""
    q = np.asarray(query, dtype=np.float32)
    k = np.asarray(key, dtype=np.float32)
    v = np.asarray(value, dtype=np.float32)
    p = np.asarray(topological_params, dtype=np.float32)

    # note: the 1/sqrt(d) query scale cancels in the normalization
    qr = np.maximum(q, 0.0) + 1e-8                # [B, L, H, D]
    kr = np.maximum(k, 0.0) + 1e-8

    # masks per head: [H, Lq, Lk]
    m_full = np.abs(p)[:, _DIST]                  # [H, L-1, L-1]
    masks = np.ones((H, L, L), np.float32)
    masks[:, 1:, 1:] = m_full

    in_maps = []
    for u in range(2):            # batch group
        for g in range(4):        # head group
            bs = slice(4 * u, 4 * u + 4)
            hs = slice(3 * g, 3 * g + 3)

            def pack_T(x):
                # [4b, L, 3h, 64] -> [3h, 64, NB*CW] (transposed, padded)
                t = x[bs, :, hs, :]                       # [4, L, 3, 64]
                t = t.transpose(2, 3, 0, 1)               # [3, 64, 4, L]
                out = np.zeros((NH, 64, NB * CW), ml_dtypes.bfloat16)
                out.reshape(NH, 64, NB, CW)[:, :, :, :L] = \
                    t.astype(ml_dtypes.bfloat16)
                return out

            if FP8:
                qk = np.zeros((NH, 32, NB, 2, 2, QW), ml_dtypes.float8_e4m3)
                qp = pack_T(qr).reshape(NH, 2, 32, NB, CW)  # [h, i, p, b, col]
                kp = pack_T(kr).reshape(NH, 2, 32, NB, CW)
                qk[:, :, :, 0, :, :CW] = qp.transpose(0, 2, 3, 1, 4)
                qk[:, :, :, 1, :, :CW] = kp.transpose(0, 2, 3, 1, 4)
                qk = qk.reshape(NH, 32, 4 * NB * QW)
            else:
                qk = np.empty((NH, 64, 2 * NB * CW), ml_dtypes.bfloat16)
                qkv4 = qk.reshape(NH, 64, NB, 2, CW)
                qkv4[:, :, :, 0, :] = pack_T(qr).reshape(NH, 64, NB, CW)
                qkv4[:, :, :, 1, :] = pack_T(kr).reshape(NH, 64, NB, CW)

            vs = v[bs, :, hs, :]                          # [4, L, 3, 64]
            v_r = np.zeros((NH, 128, NB, 5, 66), np.float16)
            for c in range(5):
                n = CNT[c]
                blk = vs[:, KS[c]:KS[c] + n].transpose(2, 1, 0, 3)
                v_r[:, :n, :, c, 0:64] = blk.astype(np.float16)
                v_r[:, :n, :, c, 64] = 1.0

            # mask tile is key-partitioned: m_r[h, key, c, q] = |M|[h, q, key]
            mT = masks[hs].transpose(0, 2, 1)             # [3, key, q]
            m_r = np.zeros((NH, 128, 5 * CW + MTW), np.float16)
            m5 = m_r[:, :, :5 * CW].reshape(NH, 128, 5, CW)
            mt = m_r[:, :, 5 * CW:].reshape(NH, 128, 5, 65)
            for c in range(5):
                n = CNT[c]
                m5[:, :n, c, :512] = mT[:, KS[c]:KS[c] + n, 0:512].astype(np.float16)
                mt[:, :n, c, :] = mT[:, KS[c]:KS[c] + n, 512:577].astype(np.float16)

            in_maps.append({
                "qk": np.ascontiguousarray(qk),
                "v": np.ascontiguousarray(v_r.reshape(NH, 128, NB * 330)),
                "m": np.ascontiguousarray(m_r),
            })
    return in_maps


def kernel(query, key, value, topological_params):
    from concourse import bass_utils
    nc = _get_nc()
    in_maps = _host_shard(query, key, value, topological_params)
    res = bass_utils.run_bass_kernel_spmd(nc, in_maps, core_ids=list(range(8)))
    out = np.empty((B, L, H, D), dtype=np.float32)
    for u in range(2):
        for g in range(4):
            o = res.results[4 * u + g]["o"]          # [3, 4, 128, 330]
            o = o.reshape(NH, NB, 128, 5, 66)
            for lc in range(5):
                lw = LW[lc]
                blk = o[:, :, 0:lw, lc, :].astype(np.float32)
                oz = blk[..., 0:64] / blk[..., 64:65]
                out[4 * u:4 * u + 4, 128 * lc:128 * lc + lw,
                    3 * g:3 * g + 3, :] = oz.transpose(1, 2, 0, 3)
    return out


# revision 3
# speedup vs baseline: 1.0345x; 1.0014x over previous
"""Trainium2 Bass kernel v2 for nn_MultiHeadDotProductAttention_24756191494231.

Masked (toeplitz-structured) linear attention:
    q = relu(query/8); k = relu(key)
    attn = (q @ k^T) * |toeplitz_mask| ; attn /= attn.sum(-1) ; out = attn @ v

Sharding: 8 cores = 2 batch-groups (4 batches) x 4 head-groups (3 heads).
Each core computes 12 (batch, head) pairs.

v2 restructure (vs v1 baseline):
  - relu/scale/cast and the |mask| gather are done on the HOST; the device
    receives bf16 q^T/k^T/v and fully materialized per-chunk mask tiles.
  - S^T[k,q] bf16 matmuls in 5 key-chunks of [128,128,128,128,65]; the
    512-wide part goes to a single-bank PSUM tile (deep rotation), the
    65-wide query tail of all 5 chunks accumulates in one shared bank and
    is masked by ONE merged TT per pair.
  - mask apply per (pair, chunk) via one of three engine paths (tunable
    METH table): Act copy->DVE 2x TT, Act copy->Pool TT, or direct DVE 1x
    TT from PSUM.
  - AV in A-orientation: out[l-chunk, 65] (64 v cols + ones col = Z).
  - One Act evacuation (PSUM->SBUF) per pair; normalize (O/Z) on the host.
"""
import sys

for _p in ("/opt/trn_rl_repo", "/root/.axon_site/_ro/trn_rl_repo"):
    if _p not in sys.path:
        sys.path.insert(0, _p)

import numpy as np
import ml_dtypes

NBX = NBY = 24
B, H, D = 8, 12, 64
L = NBX * NBY + 1          # 577
NB = 4                     # batches per core
NH = 3                     # heads per core
CNT = [128, 128, 128, 128, 65]       # key-chunk sizes
KS = [0, 128, 256, 384, 512]         # key-chunk starts
LW = [128, 128, 128, 128, 65]        # l(query)-chunk sizes
CW = 580                             # padded column stride (4B aligned bf16)
QW = 640                             # q/k block stride (chunk-4 zero padding)
MTW = 5 * 65                         # tail-mask columns

# mask-apply method per (head, batch, chunk) for the 512-wide part:
#   0 = Act copy -> DVE 2x in-place TT   (Act ~612ns, DVE ~326ns)
#   1 = Act copy -> Pool in-place TT     (Act ~612ns, Pool ~1206ns)
#   2 = direct DVE 1x TT from PSUM       (DVE ~658ns)
_PAT_A = (2, 1, 0, 1, 2)
_PAT_B = (2, 1, 0, 1, 2)
_PAT_L = (2, 1, 0, 1, 2)
METH = [[_PAT_A, _PAT_A, _PAT_A, _PAT_B] for _ in range(NH)]
METH[NH - 1][NB - 1] = _PAT_L
BUFS = dict(sb=3, sba=5, sbo=6, sa=5, st=1, po=2)
AVDEPTH = 3
EVAC_DVE = {3, 6, 9}   # pair indices whose evac runs on DVE
MODE = 'full'   # 'full' | 'skeleton' (no mask stage, AV reads m_t)
FP8 = True     # fp8e4 DoubleRow S^T matmuls (q/k in fp8, halves PE cost)
EVAC2 = False  # interp-safe evacuation (reads only written PSUM bytes)


def _av_order(pat):
    """AV accumulation order: Pool-masked (meth==1) chunks last."""
    return [c for c in range(5) if pat[c] != 1] +            [c for c in range(5) if pat[c] == 1]

_CACHE = {}


def _split_excess_waits(nc):
    """Walrus accepts at most ONE sync-wait per instruction (zero on
    Pool-engine ops). Move excess waits onto same-engine InstEventSemaphore
    instructions inserted immediately before the offending instruction."""
    import concourse.mybir as mb
    ctr = 0
    f = nc.m.functions[0]
    for bb in f.blocks:
        insts = list(bb.instructions)
        out = []
        changed = False
        for inst in insts:
            si = inst.sync_info
            keep = 0 if inst.engine == mb.EngineType.Pool else 1
            if si is not None and len(si.on_wait) > keep:
                waits = list(si.on_wait)
                moved = waits[:-keep] if keep else waits
                kept = waits[-keep:] if keep else []
                for w in moved:
                    ctr += 1
                    ev = mb.InstEventSemaphore(
                        name=f"zz_waitsplit_{ctr}", ins=[], outs=[])
                    ev.engine = inst.engine
                    ev.sync_info = mb.SyncInfo(on_wait=[w], on_update=[])
                    out.append(ev)
                inst.sync_info = mb.SyncInfo(
                    on_wait=kept, on_update=list(si.on_update))
                changed = True
            out.append(inst)
        if changed:
            bb.instructions = out


def _build_bass(split_waits=True):
    import concourse.bass as bass
    import concourse.mybir as mybir
    from concourse.bass_types import AP
    from concourse.tile import TileContext

    F32 = mybir.dt.float32
    BF16 = mybir.dt.bfloat16
    F16 = mybir.dt.float16
    Alu = mybir.AluOpType
    Act = mybir.ActivationFunctionType

    F8 = mybir.dt.float8e4
    nc = bass.Bass("TRN2")
    if FP8:
        qk_d = nc.dram_tensor("qk", (NH, 32, 4 * NB * QW), F8,
                              kind="ExternalInput")
    else:
        qk_d = nc.dram_tensor("qk", (NH, 64, 2 * NB * CW), BF16,
                              kind="ExternalInput")
    v_d = nc.dram_tensor("v", (NH, 128, NB * 330), F16, kind="ExternalInput")
    m_d = nc.dram_tensor("m", (NH, 128, 5 * CW + MTW), F16,
                         kind="ExternalInput")
    o_d = nc.dram_tensor("o", (NH, NB, 128, 330), F16,
                         kind="ExternalOutput")

    with TileContext(nc) as tc:
        with (
            tc.tile_pool(name="sb", bufs=BUFS["sb"]) as sb,
            tc.tile_pool(name="sba", bufs=BUFS["sba"]) as sba,
            tc.tile_pool(name="sbo", bufs=BUFS["sbo"]) as sbo,
            tc.tile_pool(name="ps_a", bufs=BUFS["sa"], space="PSUM") as ps_a,
            tc.tile_pool(name="ps_t", bufs=BUFS["st"], space="PSUM") as ps_t,
            tc.tile_pool(name="ps_o", bufs=BUFS["po"], space="PSUM") as ps_o,
        ):
            def load_head(h):
                if FP8:
                    qk = sb.tile([32, 4 * NB * QW], F8, tag="qk")
                    half = 2 * NB * QW
                    nc.sync.dma_start(qk[:, 0:half], qk_d[h][:, 0:half])
                    m_t = sb.tile([128, 5 * CW + MTW], F16, tag="m_t")
                    nc.sync.dma_start(m_t[:, 0:2 * CW], m_d[h][:, 0:2 * CW])
                    nc.sync.dma_start(qk[:, half:2 * half],
                                      qk_d[h][:, half:2 * half])
                    nc.sync.dma_start(m_t[:, 2 * CW:5 * CW + MTW],
                                      m_d[h][:, 2 * CW:5 * CW + MTW])
                    v_t = sb.tile([128, NB * 330], F16, tag="v_t")
                    nc.sync.dma_start(v_t, v_d[h])
                    return dict(qk=qk, v=v_t, m=m_t, h=h)
                qk = sb.tile([64, 2 * NB * CW], BF16, tag="qk")
                nc.sync.dma_start(qk[:, 0:4 * CW], qk_d[h][:, 0:4 * CW])
                m_t = sb.tile([128, 5 * CW + MTW], F16, tag="m_t")
                nc.sync.dma_start(m_t[:, 0:2 * CW], m_d[h][:, 0:2 * CW])
                nc.sync.dma_start(qk[:, 4 * CW:8 * CW],
                                  qk_d[h][:, 4 * CW:8 * CW])
                nc.sync.dma_start(m_t[:, 2 * CW:5 * CW + MTW],
                                  m_d[h][:, 2 * CW:5 * CW + MTW])
                v_t = sb.tile([128, NB * 330], F16, tag="v_t")
                nc.sync.dma_start(v_t, v_d[h])
                return dict(qk=qk, v=v_t, m=m_t, h=h)

            def av_lcgroup(R, b, a_t, o_ps, lc):
                # one sequential PSUM accumulation group (all 5 key chunks)
                lw = LW[lc]
                src_t = R["m"] if MODE == 'skeleton' else a_t
                for c in range(5):
                    cnt = CNT[c]
                    nc.tensor.matmul(
                        o_ps[0:lw, 66 * lc:66 * lc + 65],
                        src_t[0:cnt, CW * c + 128 * lc:CW * c + 128 * lc + lw],
                        R["v"][0:cnt, 330 * b + 66 * c:330 * b + 66 * c + 65],
                        start=(c == 0), stop=(c == 4))

            def evac_pair(R, b, o_ps, eng="act"):
                # fp16 output with 1/16 scale; the scale cancels in the
                # host-side O/Z divide
                o_sb = sbo.tile([128, 330], F16, tag="o_sb")
                if eng == "dve" and not EVAC2:
                    nc.vector.tensor_scalar(out=o_sb, in0=o_ps[:, 0:330],
                                            scalar1=0.0625, scalar2=None,
                                            op0=Alu.mult)
                    nc.sync.dma_start(o_d[R["h"], b], o_sb)
                    return
                if EVAC2:
                    nc.scalar.activation(
                        o_sb[:, 0:264].rearrange(
                            "p (l j) -> p l j", j=66)[:, :, 0:65],
                        o_ps[:, 0:264].rearrange(
                            "p (l j) -> p l j", j=66)[:, :, 0:65],
                        Act.Copy, scale=0.0625)
                    nc.scalar.activation(o_sb[0:65, 264:329],
                                         o_ps[0:65, 264:329], Act.Copy,
                                         scale=0.0625)
                    dst = o_d[R["h"], b]
                    nc.sync.dma_start(
                        dst[:, 0:264].rearrange(
                            "p (l j) -> p l j", j=66)[:, :, 0:65],
                        o_sb[:, 0:264].rearrange(
                            "p (l j) -> p l j", j=66)[:, :, 0:65])
                    nc.sync.dma_start(dst[0:65, 264:329],
                                      o_sb[0:65, 264:329])
                else:
                    nc.scalar.activation(o_sb, o_ps[:, 0:330], Act.Copy,
                                         scale=0.0625)
                    nc.sync.dma_start(o_d[R["h"], b], o_sb)

            # software pipeline: S^T+mask of pair i overlaps AV of pair
            # i-AVDEPTH (deeper pipelining decouples mask latency from PE)
            heads = [None, None, None]
            heads[0] = load_head(0)
            heads[1] = load_head(1)
            pending = []
            for h in range(NH):
                R = heads[h]
                for b in range(NB):
                    hh = R["h"]
                    if FP8:
                        qo = 4 * QW * b
                        ko = qo + 2 * QW
                        pitch = 4 * NB * QW
                    else:
                        qo = 2 * CW * b
                        ko = 2 * CW * b + CW
                    a_t = sba.tile([128, 5 * CW], F16, tag="a_t")
                    s_t = ps_t.tile([128, MTW], F32, tag="s_t")
                    idx = h * NB + b
                    npop = 0
                    if len(pending) >= AVDEPTH:
                        npop = 1
                    if idx >= NH * NB - (AVDEPTH - 1) and pending:
                        npop = min(2, len(pending))
                    readies = [pending.pop(0) for _ in range(npop)]
                    ready = readies[0] if readies else None
                    if ready is not None:
                        o_ps = ps_o.tile([128, 330], F32, tag="o_ps")
                    for c in range(5):
                        cnt = CNT[c]
                        s_a = ps_a.tile([128, 512], F32, tag="s_a")
                        if FP8:
                            cmm = 128 if c == 4 else cnt  # pad-keys: full M
                            qkt = R["qk"].tensor
                            lhs = AP(qkt, ko + KS[c],
                                     [[pitch, 32], [QW, 2], [1, cmm]])
                            rhs_a = AP(qkt, qo,
                                       [[pitch, 32], [QW, 2], [1, 512]])
                            rhs_t = AP(qkt, qo + 512,
                                       [[pitch, 32], [QW, 2], [1, 65]])
                            dr = mybir.MatmulPerfMode.DoubleRow
                            nc.tensor.matmul(s_a[0:cmm, 0:512], lhs, rhs_a,
                                             start=True, stop=True,
                                             perf_mode=dr)
                            nc.tensor.matmul(s_t[0:cmm, 65 * c:65 * c + 65],
                                             lhs, rhs_t,
                                             start=True, stop=True,
                                             perf_mode=dr,
                                             skip_group_check=True)
                        else:
                            lhs = R["qk"][:, ko + KS[c]:ko + KS[c] + cnt]
                            nc.tensor.matmul(s_a[0:cnt, 0:512],
                                             lhs, R["qk"][:, qo:qo + 512],
                                             start=True, stop=True)
                            nc.tensor.matmul(s_t[0:cnt, 65 * c:65 * c + 65],
                                             lhs,
                                             R["qk"][:, qo + 512:qo + 577],
                                             start=True, stop=True,
                                             skip_group_check=True)

                        if MODE == 'skeleton':
                            continue
                        meth = METH[hh][b][c]
                        ao = a_t[0:cnt, CW * c:CW * c + 512]
                        mo = R["m"][0:cnt, CW * c:CW * c + 512]
                        if meth == 2:
                            nc.vector.tensor_tensor(
                                out=ao, in0=s_a[0:cnt, 0:512], in1=mo,
                                op=Alu.mult)
                        elif meth == 3:
                            # Act copy + DMA-engine elementwise multiply
                            nc.scalar.activation(ao, s_a[0:cnt, 0:512],
                                                 Act.Copy)
                            nc.scalar.dma_start(ao, mo,
                                                accum_op=Alu.mult)
                        else:
                            nc.scalar.activation(ao, s_a[0:cnt, 0:512],
                                                 Act.Copy)
                            eng = nc.vector if meth == 0 else nc.gpsimd
                            eng.tensor_tensor(out=ao, in0=ao, in1=mo,
                                              op=Alu.mult)
                    if ready is not None:
                        pR, pb, pa = ready
                        for lc in range(5):
                            av_lcgroup(pR, pb, pa, o_ps, lc)
                    if MODE != 'skeleton':
                        # merged query-tail mask TT for all 5 chunks
                        ta = AP(a_t.tensor, 512,
                                [[5 * CW, 128], [CW, 5], [1, 65]])
                        nc.vector.tensor_tensor(
                            out=ta,
                            in0=s_t[:, :].rearrange("p (c j) -> p c j", j=65),
                            in1=R["m"][:, 5 * CW:5 * CW + MTW].rearrange(
                                "p (c j) -> p c j", j=65),
                            op=Alu.mult)
                    if ready is not None:
                        evac_pair(ready[0], ready[1], o_ps,
                                  eng="dve" if idx in EVAC_DVE else "act")
                    for extra in readies[1:]:
                        o_ps2 = ps_o.tile([128, 330], F32, tag="o_ps")
                        for lc in range(5):
                            av_lcgroup(extra[0], extra[1], extra[2], o_ps2, lc)
                        evac_pair(extra[0], extra[1], o_ps2)
                    pending.append((R, b, a_t))
                if h + 2 < NH:
                    heads[h + 2] = load_head(h + 2)
            # drain remaining pairs
            for ready in pending:
                pR, pb, pa = ready
                o_ps = ps_o.tile([128, 330], F32, tag="o_ps")
                for lc in range(5):
                    av_lcgroup(pR, pb, pa, o_ps, lc)
                evac_pair(pR, pb, o_ps)

    if split_waits:
        _split_excess_waits(nc)
    return nc


def _get_nc():
    if "nc" not in _CACHE:
        _CACHE["nc"] = _build_bass()
    return _CACHE["nc"]


def _dist_index():
    """Flattened toeplitz displacement index [L-1, L-1] into params[:, 4*NBX*NBY]."""
    gi = np.arange(NBX)
    dist = ((gi[:, None, None, None] - gi[None, None, :, None] + NBX) * 2 * NBY
            + gi[None, :, None, None] - gi[None, None, None, :] + NBY)
    return dist.reshape(NBX * NBY, NBX * NBY)


_DIST = _dist_index()


def _host_shard(query, key, value, topological_params):
    """Build the 8 per-core input dicts (slicing / relu / cast / mask)."""
    q = np.asarray(query, dtype=np.float32)
    k = np.asarray(key, dtype=np.float32)
    v = np.asarray(value, dtype=np.float32)
    p = np.asarray(topological_params, dtype=np.float32)

    # note: the 1/sqrt(d) query scale cancels in the normalization
    qr = np.maximum(q, 0.0) + 1e-8                # [B, L, H, D]
    kr = np.maximum(k, 0.0) + 1e-8

    # masks per head: [H, Lq, Lk]
    m_full = np.abs(p)[:, _DIST]                  # [H, L-1, L-1]
    masks = np.ones((H, L, L), np.float32)
    masks[:, 1:, 1:] = m_full

    in_maps = []
    for u in range(2):            # batch group
        for g in range(4):        # head group
            bs = slice(4 * u, 4 * u + 4)
            hs = slice(3 * g, 3 * g + 3)

            def pack_T(x):
                # [4b, L, 3h, 64] -> [3h, 64, NB*CW] (transposed, padded)
                t = x[bs, :, hs, :]                       # [4, L, 3, 64]
                t = t.transpose(2, 3, 0, 1)               # [3, 64, 4, L]
                out = np.zeros((NH, 64, NB * CW), ml_dtypes.bfloat16)
                out.reshape(NH, 64, NB, CW)[:, :, :, :L] = \
                    t.astype(ml_dtypes.bfloat16)
                return out

            if FP8:
                qk = np.zeros((NH, 32, NB, 2, 2, QW), ml_dtypes.float8_e4m3)
                qp = pack_T(qr).reshape(NH, 2, 32, NB, CW)  # [h, i, p, b, col]
                kp = pack_T(kr).reshape(NH, 2, 32, NB, CW)
                qk[:, :, :, 0, :, :CW] = qp.transpose(0, 2, 3, 1, 4)
                qk[:, :, :, 1, :, :CW] = kp.transpose(0, 2, 3, 1, 4)
                qk = qk.reshape(NH, 32, 4 * NB * QW)
            else:
                qk = np.empty((NH, 64, 2 * NB * CW), ml_dtypes.bfloat16)
                qkv4 = qk.reshape(NH, 64, NB, 2, CW)
                qkv4[:, :, :, 0, :] = pack_T(qr).reshape(NH, 64, NB, CW)
                qkv4[:, :, :, 1, :] = pack_T(kr).reshape(NH, 64, NB, CW)

            vs = v[bs, :, hs, :]                          # [4, L, 3, 64]
            v_r = np.zeros((NH, 128, NB, 5, 66), np.float16)
            for c in range(5):
                n = CNT[c]
                blk = vs[:, KS[c]:KS[c] + n].transpose(2, 1, 0, 3)
                v_r[:, :n, :, c, 0:64] = blk.astype(np.float16)
                v_r[:, :n, :, c, 64] = 1.0

            # mask tile is key-partitioned: m_r[h, key, c, q] = |M|[h, q, key]
            mT = masks[hs].transpose(0, 2, 1)             # [3, key, q]
            m_r = np.zeros((NH, 128, 5 * CW + MTW), np.float16)
            m5 = m_r[:, :, :5 * CW].reshape(NH, 128, 5, CW)
            mt = m_r[:, :, 5 * CW:].reshape(NH, 128, 5, 65)
            for c in range(5):
                n = CNT[c]
                m5[:, :n, c, :512] = mT[:, KS[c]:KS[c] + n, 0:512].astype(np.float16)
                mt[:, :n, c, :] = mT[:, KS[c]:KS[c] + n, 512:577].astype(np.float16)

            in_maps.append({
                "qk": np.ascontiguousarray(qk),
                "v": np.ascontiguousarray(v_r.reshape(NH, 128, NB * 330)),
                "m": np.ascontiguousarray(m_r),
            })
    return in_maps


def kernel(query, key, value, topological_params):
    from concourse import bass_utils
    nc = _get_nc()
    in_maps = _host_shard(query, key, value, topological_params)
    res = bass_utils.run_bass_kernel_spmd(nc, in_maps, core_ids=list(range(8)))
    out = np.empty((B, L, H, D), dtype=np.float32)
    for u in range(2):
        for g in range(4):
            o = res.results[4 * u + g]["o"]          # [3, 4, 128, 330]
            o = o.reshape(NH, NB, 128, 5, 66)
            for lc in range(5):
                lw = LW[lc]
                blk = o[:, :, 0:lw, lc, :].astype(np.float32)
                oz = blk[..., 0:64] / blk[..., 64:65]
                out[4 * u:4 * u + 4, 128 * lc:128 * lc + lw,
                    3 * g:3 * g + 3, :] = oz.transpose(1, 2, 0, 3)
    return out


# revision 4
# speedup vs baseline: 1.0503x; 1.0154x over previous
"""Trainium2 Bass kernel v2 for nn_MultiHeadDotProductAttention_24756191494231.

Masked (toeplitz-structured) linear attention:
    q = relu(query/8); k = relu(key)
    attn = (q @ k^T) * |toeplitz_mask| ; attn /= attn.sum(-1) ; out = attn @ v

Sharding: 8 cores = 2 batch-groups (4 batches) x 4 head-groups (3 heads).
Each core computes 12 (batch, head) pairs.

v2 restructure (vs v1 baseline):
  - relu/scale/cast and the |mask| gather are done on the HOST; the device
    receives bf16 q^T/k^T/v and fully materialized per-chunk mask tiles.
  - S^T[k,q] bf16 matmuls in 5 key-chunks of [128,128,128,128,65]; the
    512-wide part goes to a single-bank PSUM tile (deep rotation), the
    65-wide query tail of all 5 chunks accumulates in one shared bank and
    is masked by ONE merged TT per pair.
  - mask apply per (pair, chunk) via one of three engine paths (tunable
    METH table): Act copy->DVE 2x TT, Act copy->Pool TT, or direct DVE 1x
    TT from PSUM.
  - AV in A-orientation: out[l-chunk, 65] (64 v cols + ones col = Z).
  - One Act evacuation (PSUM->SBUF) per pair; normalize (O/Z) on the host.
"""
import sys

for _p in ("/opt/trn_rl_repo", "/root/.axon_site/_ro/trn_rl_repo"):
    if _p not in sys.path:
        sys.path.insert(0, _p)

import numpy as np
import ml_dtypes

NBX = NBY = 24
B, H, D = 8, 12, 64
L = NBX * NBY + 1          # 577
NB = 4                     # batches per core
NH = 3                     # heads per core
CNT = [128, 128, 128, 128, 65]       # key-chunk sizes
KS = [0, 128, 256, 384, 512]         # key-chunk starts
LW = [128, 128, 128, 128, 65]        # l(query)-chunk sizes
CW = 580                             # padded column stride (4B aligned bf16)
QW = 640                             # q/k block stride (chunk-4 zero padding)
MTW = 5 * 65                         # tail-mask columns

# mask-apply method per (head, batch, chunk) for the 512-wide part:
#   0 = Act copy -> DVE 2x in-place TT   (Act ~612ns, DVE ~326ns)
#   1 = Act copy -> Pool in-place TT     (Act ~612ns, Pool ~1206ns)
#   2 = direct DVE 1x TT from PSUM       (DVE ~658ns)
_PAT_A = (2, 1, 0, 1, 2)
METH = [[list(_PAT_A) for _ in range(NB)] for _ in range(NH)]
METH[2][1] = [2, 0, 0, 1, 2]   # one Pool unit shifted to DVE (balance)
BUFS = dict(sb=3, sba=5, sbo=8, sa=5, st=1, po=2)
AVDEPTH = 3
EVAC_DVE = {4, 7}   # pair indices whose evac runs on DVE
MODE = 'full'   # 'full' | 'skeleton' (no mask stage, AV reads m_t)
FP8 = True     # fp8e4 DoubleRow S^T matmuls (q/k in fp8, halves PE cost)
EVAC2 = False  # interp-safe evacuation (reads only written PSUM bytes)


def _av_order(pat):
    """AV accumulation order: Pool-masked (meth==1) chunks last."""
    return [c for c in range(5) if pat[c] != 1] +            [c for c in range(5) if pat[c] == 1]

_CACHE = {}


def _split_excess_waits(nc):
    """Walrus accepts at most ONE sync-wait per instruction (zero on
    Pool-engine ops). Move excess waits onto same-engine InstEventSemaphore
    instructions inserted immediately before the offending instruction."""
    import concourse.mybir as mb
    ctr = 0
    f = nc.m.functions[0]
    for bb in f.blocks:
        insts = list(bb.instructions)
        out = []
        changed = False
        for inst in insts:
            si = inst.sync_info
            keep = 0 if inst.engine == mb.EngineType.Pool else 1
            if si is not None and len(si.on_wait) > keep:
                waits = list(si.on_wait)
                moved = waits[:-keep] if keep else waits
                kept = waits[-keep:] if keep else []
                for w in moved:
                    ctr += 1
                    ev = mb.InstEventSemaphore(
                        name=f"zz_waitsplit_{ctr}", ins=[], outs=[])
                    ev.engine = inst.engine
                    ev.sync_info = mb.SyncInfo(on_wait=[w], on_update=[])
                    out.append(ev)
                inst.sync_info = mb.SyncInfo(
                    on_wait=kept, on_update=list(si.on_update))
                changed = True
            out.append(inst)
        if changed:
            bb.instructions = out


def _build_bass(split_waits=True):
    import concourse.bass as bass
    import concourse.mybir as mybir
    from concourse.bass_types import AP
    from concourse.tile import TileContext

    F32 = mybir.dt.float32
    BF16 = mybir.dt.bfloat16
    F16 = mybir.dt.float16
    Alu = mybir.AluOpType
    Act = mybir.ActivationFunctionType

    F8 = mybir.dt.float8e4
    nc = bass.Bass("TRN2")
    if FP8:
        qk_d = nc.dram_tensor("qk", (NH, 32, 4 * NB * QW), F8,
                              kind="ExternalInput")
    else:
        qk_d = nc.dram_tensor("qk", (NH, 64, 2 * NB * CW), BF16,
                              kind="ExternalInput")
    v_d = nc.dram_tensor("v", (NH, 128, NB * 330), F16, kind="ExternalInput")
    m_d = nc.dram_tensor("m", (NH, 128, 5 * CW + MTW), F16,
                         kind="ExternalInput")
    o_d = nc.dram_tensor("o", (NH, NB, 128, 330), F16,
                         kind="ExternalOutput")

    with TileContext(nc) as tc:
        with (
            tc.tile_pool(name="sb", bufs=BUFS["sb"]) as sb,
            tc.tile_pool(name="sba", bufs=BUFS["sba"]) as sba,
            tc.tile_pool(name="sbo", bufs=BUFS["sbo"]) as sbo,
            tc.tile_pool(name="ps_a", bufs=BUFS["sa"], space="PSUM") as ps_a,
            tc.tile_pool(name="ps_t", bufs=BUFS["st"], space="PSUM") as ps_t,
            tc.tile_pool(name="ps_o", bufs=BUFS["po"], space="PSUM") as ps_o,
        ):
            def load_head(h):
                if FP8:
                    qk = sb.tile([32, 4 * NB * QW], F8, tag="qk")
                    half = 2 * NB * QW
                    nc.sync.dma_start(qk[:, 0:half], qk_d[h][:, 0:half])
                    m_t = sb.tile([128, 5 * CW + MTW], F16, tag="m_t")
                    nc.sync.dma_start(m_t[:, 0:2 * CW], m_d[h][:, 0:2 * CW])
                    nc.sync.dma_start(qk[:, half:2 * half],
                                      qk_d[h][:, half:2 * half])
                    nc.sync.dma_start(m_t[:, 2 * CW:5 * CW + MTW],
                                      m_d[h][:, 2 * CW:5 * CW + MTW])
                    v_t = sb.tile([128, NB * 330], F16, tag="v_t")
                    nc.sync.dma_start(v_t, v_d[h])
                    return dict(qk=qk, v=v_t, m=m_t, h=h)
                qk = sb.tile([64, 2 * NB * CW], BF16, tag="qk")
                nc.sync.dma_start(qk[:, 0:4 * CW], qk_d[h][:, 0:4 * CW])
                m_t = sb.tile([128, 5 * CW + MTW], F16, tag="m_t")
                nc.sync.dma_start(m_t[:, 0:2 * CW], m_d[h][:, 0:2 * CW])
                nc.sync.dma_start(qk[:, 4 * CW:8 * CW],
                                  qk_d[h][:, 4 * CW:8 * CW])
                nc.sync.dma_start(m_t[:, 2 * CW:5 * CW + MTW],
                                  m_d[h][:, 2 * CW:5 * CW + MTW])
                v_t = sb.tile([128, NB * 330], F16, tag="v_t")
                nc.sync.dma_start(v_t, v_d[h])
                return dict(qk=qk, v=v_t, m=m_t, h=h)

            def av_lcgroup(R, b, a_t, o_ps, lc):
                # one sequential PSUM accumulation group (all 5 key chunks)
                lw = LW[lc]
                src_t = R["m"] if MODE == 'skeleton' else a_t
                for c in range(5):
                    cnt = CNT[c]
                    nc.tensor.matmul(
                        o_ps[0:lw, 66 * lc:66 * lc + 65],
                        src_t[0:cnt, CW * c + 128 * lc:CW * c + 128 * lc + lw],
                        R["v"][0:cnt, 330 * b + 66 * c:330 * b + 66 * c + 65],
                        start=(c == 0), stop=(c == 4))

            def evac_pair(R, b, o_ps, eng="act"):
                # fp16 output with 1/16 scale; the scale cancels in the
                # host-side O/Z divide
                o_sb = sbo.tile([128, 330], F16, tag="o_sb")
                if eng == "dve" and not EVAC2:
                    nc.vector.tensor_scalar(out=o_sb, in0=o_ps[:, 0:330],
                                            scalar1=0.0625, scalar2=None,
                                            op0=Alu.mult)
                    nc.sync.dma_start(o_d[R["h"], b], o_sb)
                    return
                if EVAC2:
                    nc.scalar.activation(
                        o_sb[:, 0:264].rearrange(
                            "p (l j) -> p l j", j=66)[:, :, 0:65],
                        o_ps[:, 0:264].rearrange(
                            "p (l j) -> p l j", j=66)[:, :, 0:65],
                        Act.Copy, scale=0.0625)
                    nc.scalar.activation(o_sb[0:65, 264:329],
                                         o_ps[0:65, 264:329], Act.Copy,
                                         scale=0.0625)
                    dst = o_d[R["h"], b]
                    nc.sync.dma_start(
                        dst[:, 0:264].rearrange(
                            "p (l j) -> p l j", j=66)[:, :, 0:65],
                        o_sb[:, 0:264].rearrange(
                            "p (l j) -> p l j", j=66)[:, :, 0:65])
                    nc.sync.dma_start(dst[0:65, 264:329],
                                      o_sb[0:65, 264:329])
                else:
                    nc.scalar.activation(o_sb, o_ps[:, 0:330], Act.Copy,
                                         scale=0.0625)
                    nc.sync.dma_start(o_d[R["h"], b], o_sb)

            # software pipeline: S^T+mask of pair i overlaps AV of pair
            # i-AVDEPTH (deeper pipelining decouples mask latency from PE)
            heads = [None, None, None]
            heads[0] = load_head(0)
            heads[1] = load_head(1)
            pending = []
            for h in range(NH):
                R = heads[h]
                for b in range(NB):
                    hh = R["h"]
                    if FP8:
                        qo = 4 * QW * b
                        ko = qo + 2 * QW
                        pitch = 4 * NB * QW
                    else:
                        qo = 2 * CW * b
                        ko = 2 * CW * b + CW
                    a_t = sba.tile([128, 5 * CW], F16, tag="a_t")
                    s_t = ps_t.tile([128, MTW], F32, tag="s_t")
                    idx = h * NB + b
                    npop = 0
                    if len(pending) >= AVDEPTH:
                        npop = 1
                    if idx >= NH * NB - (AVDEPTH - 1) and pending:
                        npop = min(2, len(pending))
                    readies = [pending.pop(0) for _ in range(npop)]
                    ready = readies[0] if readies else None
                    if ready is not None:
                        o_ps = ps_o.tile([128, 330], F32, tag="o_ps")
                    for c in range(5):
                        cnt = CNT[c]
                        s_a = ps_a.tile([128, 512], F32, tag="s_a")
                        if FP8:
                            cmm = 128 if c == 4 else cnt  # pad-keys: full M
                            qkt = R["qk"].tensor
                            lhs = AP(qkt, ko + KS[c],
                                     [[pitch, 32], [QW, 2], [1, cmm]])
                            rhs_a = AP(qkt, qo,
                                       [[pitch, 32], [QW, 2], [1, 512]])
                            rhs_t = AP(qkt, qo + 512,
                                       [[pitch, 32], [QW, 2], [1, 65]])
                            dr = mybir.MatmulPerfMode.DoubleRow
                            nc.tensor.matmul(s_a[0:cmm, 0:512], lhs, rhs_a,
                                             start=True, stop=True,
                                             perf_mode=dr)
                            nc.tensor.matmul(s_t[0:cmm, 65 * c:65 * c + 65],
                                             lhs, rhs_t,
                                             start=True, stop=True,
                                             perf_mode=dr,
                                             skip_group_check=True)
                        else:
                            lhs = R["qk"][:, ko + KS[c]:ko + KS[c] + cnt]
                            nc.tensor.matmul(s_a[0:cnt, 0:512],
                                             lhs, R["qk"][:, qo:qo + 512],
                                             start=True, stop=True)
                            nc.tensor.matmul(s_t[0:cnt, 65 * c:65 * c + 65],
                                             lhs,
                                             R["qk"][:, qo + 512:qo + 577],
                                             start=True, stop=True,
                                             skip_group_check=True)

                        if MODE == 'skeleton':
                            continue
                        meth = METH[hh][b][c]
                        ao = a_t[0:cnt, CW * c:CW * c + 512]
                        mo = R["m"][0:cnt, CW * c:CW * c + 512]
                        if meth == 2:
                            nc.vector.tensor_tensor(
                                out=ao, in0=s_a[0:cnt, 0:512], in1=mo,
                                op=Alu.mult)
                        elif meth == 3:
                            # Act copy + DMA-engine elementwise multiply
                            nc.scalar.activation(ao, s_a[0:cnt, 0:512],
                                                 Act.Copy)
                            nc.scalar.dma_start(ao, mo,
                                                accum_op=Alu.mult)
                        else:
                            nc.scalar.activation(ao, s_a[0:cnt, 0:512],
                                                 Act.Copy)
                            eng = nc.vector if meth == 0 else nc.gpsimd
                            eng.tensor_tensor(out=ao, in0=ao, in1=mo,
                                              op=Alu.mult)
                    if ready is not None:
                        pR, pb, pa = ready
                        for lc in range(5):
                            av_lcgroup(pR, pb, pa, o_ps, lc)
                    if MODE != 'skeleton':
                        # merged query-tail mask TT for all 5 chunks
                        ta = AP(a_t.tensor, 512,
                                [[5 * CW, 128], [CW, 5], [1, 65]])
                        nc.vector.tensor_tensor(
                            out=ta,
                            in0=s_t[:, :].rearrange("p (c j) -> p c j", j=65),
                            in1=R["m"][:, 5 * CW:5 * CW + MTW].rearrange(
                                "p (c j) -> p c j", j=65),
                            op=Alu.mult)
                    if ready is not None:
                        evac_pair(ready[0], ready[1], o_ps,
                                  eng="dve" if idx in EVAC_DVE else "act")
                    for extra in readies[1:]:
                        o_ps2 = ps_o.tile([128, 330], F32, tag="o_ps")
                        for lc in range(5):
                            av_lcgroup(extra[0], extra[1], extra[2], o_ps2, lc)
                        evac_pair(extra[0], extra[1], o_ps2)
                    pending.append((R, b, a_t))
                if h + 2 < NH:
                    heads[h + 2] = load_head(h + 2)
            # drain remaining pairs
            for ready in pending:
                pR, pb, pa = ready
                o_ps = ps_o.tile([128, 330], F32, tag="o_ps")
                for lc in range(5):
                    av_lcgroup(pR, pb, pa, o_ps, lc)
                evac_pair(pR, pb, o_ps)

    if split_waits:
        _split_excess_waits(nc)
    return nc


def _get_nc():
    if "nc" not in _CACHE:
        _CACHE["nc"] = _build_bass()
    return _CACHE["nc"]


def _dist_index():
    """Flattened toeplitz displacement index [L-1, L-1] into params[:, 4*NBX*NBY]."""
    gi = np.arange(NBX)
    dist = ((gi[:, None, None, None] - gi[None, None, :, None] + NBX) * 2 * NBY
            + gi[None, :, None, None] - gi[None, None, None, :] + NBY)
    return dist.reshape(NBX * NBY, NBX * NBY)


_DIST = _dist_index()


def _host_shard(query, key, value, topological_params):
    """Build the 8 per-core input dicts (slicing / relu / cast / mask)."""
    q = np.asarray(query, dtype=np.float32)
    k = np.asarray(key, dtype=np.float32)
    v = np.asarray(value, dtype=np.float32)
    p = np.asarray(topological_params, dtype=np.float32)

    # note: the 1/sqrt(d) query scale cancels in the normalization
    qr = np.maximum(q, 0.0) + 1e-8                # [B, L, H, D]
    kr = np.maximum(k, 0.0) + 1e-8

    # masks per head: [H, Lq, Lk]
    m_full = np.abs(p)[:, _DIST]                  # [H, L-1, L-1]
    masks = np.ones((H, L, L), np.float32)
    masks[:, 1:, 1:] = m_full

    in_maps = []
    for u in range(2):            # batch group
        for g in range(4):        # head group
            bs = slice(4 * u, 4 * u + 4)
            hs = slice(3 * g, 3 * g + 3)

            def pack_T(x):
                # [4b, L, 3h, 64] -> [3h, 64, NB*CW] (transposed, padded)
                t = x[bs, :, hs, :]                       # [4, L, 3, 64]
                t = t.transpose(2, 3, 0, 1)               # [3, 64, 4, L]
                out = np.zeros((NH, 64, NB * CW), ml_dtypes.bfloat16)
                out.reshape(NH, 64, NB, CW)[:, :, :, :L] = \
                    t.astype(ml_dtypes.bfloat16)
                return out

            if FP8:
                qk = np.zeros((NH, 32, NB, 2, 2, QW), ml_dtypes.float8_e4m3)
                qp = pack_T(qr).reshape(NH, 2, 32, NB, CW)  # [h, i, p, b, col]
                kp = pack_T(kr).reshape(NH, 2, 32, NB, CW)
                qk[:, :, :, 0, :, :CW] = qp.transpose(0, 2, 3, 1, 4)
                qk[:, :, :, 1, :, :CW] = kp.transpose(0, 2, 3, 1, 4)
                qk = qk.reshape(NH, 32, 4 * NB * QW)
            else:
                qk = np.empty((NH, 64, 2 * NB * CW), ml_dtypes.bfloat16)
                qkv4 = qk.reshape(NH, 64, NB, 2, CW)
                qkv4[:, :, :, 0, :] = pack_T(qr).reshape(NH, 64, NB, CW)
                qkv4[:, :, :, 1, :] = pack_T(kr).reshape(NH, 64, NB, CW)

            vs = v[bs, :, hs, :]                          # [4, L, 3, 64]
            v_r = np.zeros((NH, 128, NB, 5, 66), np.float16)
            for c in range(5):
                n = CNT[c]
                blk = vs[:, KS[c]:KS[c] + n].transpose(2, 1, 0, 3)
                v_r[:, :n, :, c, 0:64] = blk.astype(np.float16)
                v_r[:, :n, :, c, 64] = 1.0

            # mask tile is key-partitioned: m_r[h, key, c, q] = |M|[h, q, key]
            mT = masks[hs].transpose(0, 2, 1)             # [3, key, q]
            m_r = np.zeros((NH, 128, 5 * CW + MTW), np.float16)
            m5 = m_r[:, :, :5 * CW].reshape(NH, 128, 5, CW)
            mt = m_r[:, :, 5 * CW:].reshape(NH, 128, 5, 65)
            for c in range(5):
                n = CNT[c]
                m5[:, :n, c, :512] = mT[:, KS[c]:KS[c] + n, 0:512].astype(np.float16)
                mt[:, :n, c, :] = mT[:, KS[c]:KS[c] + n, 512:577].astype(np.float16)

            in_maps.append({
                "qk": np.ascontiguousarray(qk),
                "v": np.ascontiguousarray(v_r.reshape(NH, 128, NB * 330)),
                "m": np.ascontiguousarray(m_r),
            })
    return in_maps


def kernel(query, key, value, topological_params):
    from concourse import bass_utils
    nc = _get_nc()
    in_maps = _host_shard(query, key, value, topological_params)
    res = bass_utils.run_bass_kernel_spmd(nc, in_maps, core_ids=list(range(8)))
    out = np.empty((B, L, H, D), dtype=np.float32)
    for u in range(2):
        for g in range(4):
            o = res.results[4 * u + g]["o"]          # [3, 4, 128, 330]
            o = o.reshape(NH, NB, 128, 5, 66)
            for lc in range(5):
                lw = LW[lc]
                blk = o[:, :, 0:lw, lc, :].astype(np.float32)
                oz = blk[..., 0:64] / blk[..., 64:65]
                out[4 * u:4 * u + 4, 128 * lc:128 * lc + lw,
                    3 * g:3 * g + 3, :] = oz.transpose(1, 2, 0, 3)
    return out
